# revision 19
# baseline (speedup 1.0000x reference)
"""Trainium2 Bass kernel for a 2-layer LSTM + fc head.

Strategy (v2): LAYER-PIPELINED across core pairs. Cores 0-3 run layer 0
(32 batch rows each), cores 4-7 run layer 1 for the same rows, lagged
one TB-step block. y0 history blocks travel c -> c+4 once per block via
a pair AllGather (rank-0 slot delivery), hidden under the one-block lag.
This halves the PE instruction count per core per step: the recurrence
is issue-bound (~60ns per LDWEIGHTS+MATMUL pair regardless of free dim
16/32/64), so per-step cost scales with instruction count, not batch.

Recurrent weights are fp8(e3m4) scaled by 2^WSH (FWL loads 4B/cycle);
input-projection weights/biases carry the same scale; the gate
activations undo it via their input-scale operand. h stays bf16.

Both roles execute one shared SPMD program; the only divergent pieces
(x-vs-received fill of the xg rhs buffer R) sit in tc.If(pid<4) blocks.
Layer-1 cores process a garbage block 0 (R zeroed -> gates from bias
only); the resulting ~1e-2 state perturbation decays via the forget
gate over 512 real steps.

Step internals split the gates into two H-halves so the activation
chain of half A overlaps the PE work of half B and of the next step.
m-tile order: [i0 i1 f0 f1 o0 o1 g0 g1 | i2 i3 f2 f3 o2 o3 g2 g3] so
each half's sigmoid (i,f,o) and tanh (g) slabs are contiguous.
"""

import numpy as np
import ml_dtypes
import concourse.bass as bass
import concourse.bacc as bacc
import concourse.mybir as mybir
from concourse.bass_utils import run_bass_kernel_spmd
from concourse.tile import TileContext

F32 = mybir.dt.float32
BF16 = mybir.dt.bfloat16
FP8 = mybir.dt.float8e3
AF = mybir.ActivationFunctionType
BF16NP = ml_dtypes.bfloat16
FP8NP = ml_dtypes.float8_e3m4

B, T, D, H = 128, 512, 256, 512
G = 4 * H
NC = 8
PB = 32            # batch rows per core pair
TB = 32            # timesteps per block
NITER = T // TB + 2  # 18: layer-1 role lags two blocks (hides AllGather)
WSH = 7
WSCL = float(2 ** WSH)
INV_WSCL = 1.0 / WSCL

# m-tile order (source 128-row blocks, PyTorch gate order i=0-3 f=4-7
# g=8-11 o=12-15): per H-half [i i f f o o g g]
M_SRC = [0, 1, 4, 5, 12, 13, 8, 9, 2, 3, 6, 7, 14, 15, 10, 11]
RG = [[0, 4], [1, 5], [2, 6], [3, 7]]


def _build(nc):
    whhT = nc.declare_dram_parameter("whhT", [128, 64 * 128], FP8, isOutput=False)
    wihT = nc.declare_dram_parameter("wihT", [128, 64 * 128], BF16, isOutput=False)
    br = nc.declare_dram_parameter("br", [128, 16], F32, isOutput=False)
    fcwT = nc.declare_dram_parameter("fcwT", [128, 4], BF16, isOutput=False)
    # x, host-transposed: [128, (kd, t, b)] kd=4 (zero-padded for L0)
    xTd = nc.declare_dram_parameter("xT", [128, 4 * NITER * TB * PB], BF16,
                                    isOutput=False)
    out = nc.declare_dram_parameter("out", [2 * PB, 1], F32, isOutput=True)

    BLK = 4 * TB * PB  # cols per (k,t,b) block
    cc_in = nc.dram_tensor("cc_in", [3, 128, BLK], BF16, kind="Internal")
    cc_out = nc.dram_tensor("cc_out", [3, 128, 2 * BLK], BF16, kind="Internal")

    with TileContext(nc) as tc:
        with tc.tile_pool(name="wts", bufs=1) as wpool, \
             tc.tile_pool(name="stage", bufs=2) as stpool, \
             tc.tile_pool(name="work", bufs=3) as spool, \
             tc.tile_pool(name="state", bufs=3) as hpool, \
             tc.tile_pool(name="evp", bufs=2) as evpool, \
             tc.tile_pool(name="ps_g", bufs=2, space="PSUM") as ps_g, \
             tc.tile_pool(name="ps_big", bufs=2, space="PSUM") as ps_big, \
             tc.tile_pool(name="ps_fc", bufs=1, space="PSUM") as ps_fc:

            whh = wpool.tile([128, 64 * 128], FP8, tag="whh")
            nc.sync.dma_start(out=whh[:, :], in_=whhT[:, :])
            wih = wpool.tile([128, 64 * 128], BF16, tag="wih")
            nc.sync.dma_start(out=wih[:, :], in_=wihT[:, :])
            fcw_raw = stpool.tile([128, 4], BF16, tag="fcwraw")
            nc.sync.dma_start(out=fcw_raw[:, :], in_=fcwT[:, :])
            fcw_sb = wpool.tile([128, 4], BF16, tag="fcwf")
            nc.vector.tensor_copy(fcw_sb[:, :], fcw_raw[:, :])
            braw = stpool.tile([128, 16], F32, tag="braw")
            nc.sync.dma_start(out=braw[:, :], in_=br[:, :])
            b_sb = wpool.tile([128, 16], F32, tag="bf")
            nc.vector.tensor_copy(b_sb[:, :], braw[:, :])

            # xg rhs double buffer + y history double buffer
            R = wpool.tile([128, 2 * BLK], BF16, tag="Rbuf")
            ybl = wpool.tile([128, 2 * BLK], BF16, tag="ybl")
            nc.vector.memzero(R[:, :])

            pid = nc.partition_id()

            def wtile(wsb, k, m):
                return wsb[:, (k * 16 + m) * 128:(k * 16 + m) * 128 + 128]

            def rk(j, k):
                """R[j%2] k-chunk [128, TB*PB]."""
                off = (j % 2) * BLK + k * TB * PB
                return R[:, off:off + TB * PB]

            def yslot(j, k, t):
                """ybl[j%2] (k, t) h-slot [128, PB]."""
                off = (j % 2) * BLK + (k * TB + t) * PB
                return ybl[:, off:off + PB]

            # ---- xg GEMM unit: one (m, half-N) piece of a block ----
            HN = TB * PB // 2  # 512 = one PSUM bank

            def xg_unit(j, evb, u):
                m, h2 = u // 2, u % 2
                ps = ps_big.tile([128, HN], F32, tag="ps_gemm", name="psg")
                for k in range(4):
                    nc.tensor.matmul(
                        ps[:, :], lhsT=wtile(wih, k, m),
                        rhs=rk(j, k)[:, h2 * HN:(h2 + 1) * HN],
                        start=(k == 0), stop=(k == 3))
                nc.vector.tensor_scalar_add(
                    evb[:, m * TB * PB + h2 * HN:
                        m * TB * PB + (h2 + 1) * HN],
                    ps[:, :], b_sb[:, m:m + 1])

            def new_evb():
                return evpool.tile([128, 16 * TB * PB], BF16, tag="evb",
                                   name="evb")

            # ---- one recurrence step, H-half pipelined ----
            # returns new c tiles (one per half)
            def step(j, t, evb, h_prev, c_cur, first):
                c_new = [None, None]
                for hf in range(2):
                    ms = hf * 8  # m-tile base of this half
                    if not first:
                        gp = ps_g.tile([128, 8 * PB], F32, tag=f"gp{hf}",
                                       name="gp")
                        for k in range(4):
                            for mi in range(8):
                                nc.tensor.matmul(
                                    gp[:, mi * PB:(mi + 1) * PB],
                                    lhsT=wtile(whh, k, ms + mi),
                                    rhs=h_prev[k],
                                    start=(k == 0), stop=(k == 3))
                        s_pre = spool.tile([128, 8 * PB], F32, tag=f"sp{hf}",
                                           name="spre")
                        xg_ap = evb[:, :].rearrange(
                            "p (m t b) -> p m t b", m=16, t=TB)[:, ms:ms + 8, t, :]
                        nc.vector.tensor_add(
                            s_pre[:, :].rearrange("p (m b) -> p m b", m=8),
                            gp[:, :].rearrange("p (m b) -> p m b", m=8), xg_ap)
                        src_sig = s_pre[:, :]
                        rr = False
                    else:
                        xg4 = evb[:, :].rearrange(
                            "p (m t b) -> p m t b", m=16, t=TB)
                        src_sig = xg4[:, ms:ms + 8, t, :]
                        rr = True
                    # one sigmoid for all 8 m-tiles; the g-gate rows carry an
                    # extra 2x in their weights so slot u = sig(2g) and
                    # tanh(g) = 2u - 1
                    s_sig = spool.tile([128, 8 * PB], BF16, tag=f"ss{hf}",
                                       name="ssig")
                    nc.scalar.activation(
                        s_sig[:, :].rearrange("p (m b) -> p m b", m=8)
                        if rr else s_sig[:, :],
                        src_sig, AF.Sigmoid, scale=INV_WSCL)
                    s_tg = spool.tile([128, 2 * PB], BF16, tag=f"st{hf}",
                                      name="stg")
                    nc.vector.tensor_scalar(
                        s_tg[:, :], s_sig[:, 6 * PB:], 2.0, -1.0,
                        mybir.AluOpType.mult, mybir.AluOpType.add)
                    tmp = spool.tile([128, 2 * PB], BF16, tag=f"tm{hf}",
                                     name="tmp")
                    nc.vector.tensor_mul(tmp[:, :], s_sig[:, :2 * PB],
                                         s_tg[:, :])
                    cn = hpool.tile([128, 2 * PB], F32, tag=f"c{hf}", name="cn")
                    if not first:
                        nc.vector.tensor_mul(cn[:, :], s_sig[:, 2 * PB:4 * PB],
                                             c_cur[hf][:, :])
                        nc.vector.tensor_add(cn[:, :], cn[:, :], tmp[:, :])
                    else:
                        nc.vector.tensor_copy(cn[:, :], tmp[:, :])
                    s_tc = spool.tile([128, 2 * PB], BF16, tag=f"sc{hf}",
                                      name="stc")
                    nc.scalar.activation(s_tc[:, :], cn[:, :], AF.Tanh)
                    # h chunks 2hf, 2hf+1 -> ybl slots (strided 2-chunk view)
                    hdst = ybl[:, :].rearrange(
                        "p (s k t b) -> p s k t b", s=2, k=4, t=TB)[
                        :, j % 2, 2 * hf:2 * hf + 2, t, :]
                    nc.vector.tensor_mul(hdst, s_sig[:, 4 * PB:6 * PB]
                                         .rearrange("p (m b) -> p m b", m=2),
                                         s_tc[:, :]
                                         .rearrange("p (m b) -> p m b", m=2))
                    c_new[hf] = cn
                return c_new

            # ---- main loop over blocks; the xg GEMM of block j+1 is
            # interleaved one (m, half) unit per step of block j so the PE
            # has independent work while step activation chains resolve ----
            def fill_R(j):
                with tc.If(pid < 4) as cmp:
                    # layer-0 role: fill R[j%2] with x block j
                    nc.sync.dma_start(
                        out=R[:, (j % 2) * BLK:(j % 2 + 1) * BLK],
                        in_=xTd[:, :].rearrange(
                            "p (k t b) -> p k t b", k=4, t=NITER * TB)
                        [:, :, j * TB:(j + 1) * TB, :])
                with cmp.Else():
                    if j >= 2:
                        # layer-1 role: fill R[j%2] with partner y0 block j-2
                        nc.sync.dma_start(
                            out=R[:, (j % 2) * BLK:(j % 2 + 1) * BLK],
                            in_=cc_out[(j - 2) % 3, :, :BLK])

            c_cur = None
            hT0 = stpool.tile([128, 4 * PB], BF16, tag="hT0")
            fill_R(0)
            evb_cur = new_evb()
            for u in range(32):
                xg_unit(0, evb_cur, u)
            for j in range(NITER):
                if j == NITER - 2:
                    # snapshot layer-0 role's final h (end of data block 15)
                    # before iteration 17 overwrites that ybl parity slot
                    nc.vector.tensor_copy(
                        hT0[:, :].rearrange("p (k b) -> p k b", k=4),
                        ybl[:, :].rearrange("p (s k t b) -> p s k t b",
                                            s=2, k=4, t=TB)
                        [:, (NITER - 3) % 2, :, TB - 1, :])
                evb_next = None
                if j + 1 < NITER:
                    fill_R(j + 1)
                    evb_next = new_evb()
                for tt in range(TB):
                    tprog = j * TB + tt
                    if tprog == 0:
                        h_prev = None
                    elif tt == 0:
                        h_prev = [yslot(j - 1, k, TB - 1) for k in range(4)]
                    else:
                        h_prev = [yslot(j, k, tt - 1) for k in range(4)]
                    c_cur = step(j, tt, evb_cur, h_prev, c_cur, tprog == 0)
                    if evb_next is not None:
                        # next block's xg piece trails the step so the
                        # critical-path ops lead each engine's queue
                        xg_unit(j + 1, evb_next, tt)
                evb_cur = evb_next
                if j < NITER - 2:
                    nc.sync.dma_start(
                        out=cc_in[j % 3, :, :],
                        in_=ybl[:, (j % 2) * BLK:(j % 2 + 1) * BLK])
                    nc.gpsimd.collective_compute(
                        "AllGather", mybir.AluOpType.bypass,
                        ins=[cc_in[j % 3, :, :]],
                        outs=[cc_out[j % 3, :, :]],
                        replica_groups=RG)

            # ---- fc head: candidate final h: layer-0 role from the block-15
            # snapshot, layer-1 role from its last iteration; host selects ----
            hT1 = stpool.tile([128, 4 * PB], BF16, tag="hT1")
            nc.vector.tensor_copy(
                hT1[:, :].rearrange("p (k b) -> p k b", k=4),
                ybl[:, :].rearrange("p (s k t b) -> p s k t b",
                                    s=2, k=4, t=TB)
                [:, (NITER - 1) % 2, :, TB - 1, :])
            for li, hT in ((0, hT0), (1, hT1)):
                ps = ps_fc.tile([PB, 1], F32, tag="ps_fc", name="psfc")
                for k in range(4):
                    nc.tensor.matmul(ps[:, :], lhsT=hT[:, k * PB:(k + 1) * PB],
                                     rhs=fcw_sb[:, k:k + 1],
                                     start=(k == 0), stop=(k == 3))
                ov = spool.tile([PB, 1], F32, tag="ov", name="ov")
                nc.vector.tensor_scalar_add(ov[:, :], ps[:, :], 30.0)
                nc.sync.dma_start(out=out[li * PB:(li + 1) * PB, :],
                                  in_=ov[:, :])
    return nc


_cache = {}


def build_kernel():
    if "k" not in _cache:
        nc = bacc.Bacc("TRN2", target_bir_lowering=False, debug=False,
                       num_devices=NC)
        _build(nc)
        nc.compile()
        _cache["k"] = nc
    return _cache["k"]


def _wT_host(w, dtype):
    """w [G, 512] f32 (zero-padded cols if needed) -> [128, 64*128];
    block (k,m) = w[M_SRC[m]*128:+128, k*128:+128].T"""
    outw = np.empty((128, 64 * 128), dtype=dtype)
    for k in range(4):
        for m in range(16):
            blk = w[M_SRC[m] * 128:(M_SRC[m] + 1) * 128,
                    k * 128:(k + 1) * 128].T
            outw[:, (k * 16 + m) * 128:(k * 16 + m + 1) * 128] = blk.astype(dtype)
    return outw


def _prep_role(w_ih, w_hh, bb):
    """Per-role (layer) weight staging; w_ih padded to 512 cols. The
    g-gate rows (2H:3H) get an extra 2x so the single sigmoid computes
    u = sig(2g) and tanh(g) = 2u - 1 in a cheap fused scalar op."""
    kin = w_ih.shape[1]
    wi = np.zeros((G, 512), np.float32)
    wi[:, :kin] = w_ih.astype(np.float32) * WSCL
    wh = w_hh.astype(np.float32) * WSCL
    b = bb.astype(np.float32).reshape(G) * WSCL
    wi[2 * H:3 * H] *= 2.0
    wh = wh.copy()
    wh[2 * H:3 * H] *= 2.0
    b = b.copy()
    b[2 * H:3 * H] *= 2.0
    brr = np.stack([b[M_SRC[m] * 128:(M_SRC[m] + 1) * 128]
                    for m in range(16)], 1)
    return {
        "whhT": _wT_host(wh, FP8NP),
        "wihT": _wT_host(wi, BF16NP),
        "br": np.ascontiguousarray(brr),
    }


def run(inputs, **kw):
    nc = build_kernel()
    x = inputs["x"].astype(np.float32)
    fcw = inputs["fc_w"].astype(np.float32).reshape(H)
    fcm = np.ascontiguousarray(fcw.reshape(4, 128).T.astype(BF16NP))
    role0 = _prep_role(inputs["w_ih0"], inputs["w_hh0"], inputs["b0"])
    role1 = _prep_role(inputs["w_ih1"], inputs["w_hh1"], inputs["b1"])
    xz = np.zeros((128, 4 * NITER * TB * PB), BF16NP)
    in_maps = []
    for c in range(NC):
        role = role0 if c < 4 else role1
        m = dict(role)
        m["fcwT"] = fcm
        if c < 4:
            xs = x[c * PB:(c + 1) * PB]              # [PB, T, D]
            xt = np.zeros((128, 4, NITER * TB, PB), np.float32)
            xsw = xs.reshape(PB, T, 2, 128).transpose(3, 2, 1, 0)
            xt[:, :2, :T, :] = xsw
            m["xT"] = np.ascontiguousarray(
                xt.reshape(128, 4 * NITER * TB * PB)).astype(BF16NP)
        else:
            m["xT"] = xz
        in_maps.append(m)
    res = run_bass_kernel_spmd(nc, in_maps, core_ids=list(range(NC)), **kw)
    outp = np.zeros((2 * B, 1), np.float32)
    for c in range(NC):
        r = res.results[c]["out"]
        if c < 4:
            outp[c * PB:(c + 1) * PB] = r[:PB]       # layer-0 hn rows
        else:
            cc = c - 4
            outp[B + cc * PB:B + (cc + 1) * PB] = r[PB:]  # layer-1 rows
    return outp, res


def kernel(**inputs):
    outp, _ = run(inputs)
    return outp


# revision 21
# speedup vs baseline: 1.0156x; 1.0156x over previous
"""Trainium2 Bass kernel for a 2-layer LSTM + fc head.

Strategy (v2): LAYER-PIPELINED across core pairs. Cores 0-3 run layer 0
(32 batch rows each), cores 4-7 run layer 1 for the same rows, lagged
one TB-step block. y0 history blocks travel c -> c+4 once per block via
a pair AllGather (rank-0 slot delivery), hidden under the one-block lag.
This halves the PE instruction count per core per step: the recurrence
is issue-bound (~60ns per LDWEIGHTS+MATMUL pair regardless of free dim
16/32/64), so per-step cost scales with instruction count, not batch.

Recurrent weights are fp8(e3m4) scaled by 2^WSH (FWL loads 4B/cycle);
input-projection weights/biases carry the same scale; the gate
activations undo it via their input-scale operand. h stays bf16.

Both roles execute one shared SPMD program; the only divergent pieces
(x-vs-received fill of the xg rhs buffer R) sit in tc.If(pid<4) blocks.
Layer-1 cores process a garbage block 0 (R zeroed -> gates from bias
only); the resulting ~1e-2 state perturbation decays via the forget
gate over 512 real steps.

Step internals split the gates into two H-halves so the activation
chain of half A overlaps the PE work of half B and of the next step.
m-tile order: [i0 i1 f0 f1 o0 o1 g0 g1 | i2 i3 f2 f3 o2 o3 g2 g3] so
each half's sigmoid (i,f,o) and tanh (g) slabs are contiguous.
"""

import numpy as np
import ml_dtypes
import concourse.bass as bass
import concourse.bacc as bacc
import concourse.mybir as mybir
from concourse.bass_utils import run_bass_kernel_spmd
from concourse.tile import TileContext

F32 = mybir.dt.float32
BF16 = mybir.dt.bfloat16
FP8 = mybir.dt.float8e3
AF = mybir.ActivationFunctionType
BF16NP = ml_dtypes.bfloat16
FP8NP = ml_dtypes.float8_e3m4

B, T, D, H = 128, 512, 256, 512
G = 4 * H
NC = 8
PB = 32            # batch rows per core pair
TB = 32            # timesteps per block
NITER = T // TB + 2  # 18: layer-1 role lags two blocks (hides AllGather)
WSH = 7
WSCL = float(2 ** WSH)
INV_WSCL = 1.0 / WSCL

# m-tile order (source 128-row blocks, PyTorch gate order i=0-3 f=4-7
# g=8-11 o=12-15): per H-half [i i f f o o g g]
M_SRC = [0, 1, 4, 5, 12, 13, 8, 9, 2, 3, 6, 7, 14, 15, 10, 11]
RG = [[0, 4], [1, 5], [2, 6], [3, 7]]


def _build(nc):
    whhT = nc.declare_dram_parameter("whhT", [128, 64 * 128], FP8, isOutput=False)
    wihT = nc.declare_dram_parameter("wihT", [128, 64 * 128], BF16, isOutput=False)
    br = nc.declare_dram_parameter("br", [128, 16], F32, isOutput=False)
    fcwT = nc.declare_dram_parameter("fcwT", [128, 4], BF16, isOutput=False)
    # x, host-transposed: [128, (kd, t, b)] kd=4 (zero-padded for L0)
    xTd = nc.declare_dram_parameter("xT", [128, 4 * NITER * TB * PB], BF16,
                                    isOutput=False)
    out = nc.declare_dram_parameter("out", [2 * PB, 1], F32, isOutput=True)

    BLK = 4 * TB * PB  # cols per (k,t,b) block
    cc_in = nc.dram_tensor("cc_in", [3, 128, BLK], BF16, kind="Internal")
    cc_out = nc.dram_tensor("cc_out", [3, 128, 2 * BLK], BF16, kind="Internal")

    with TileContext(nc) as tc:
        with tc.tile_pool(name="wts", bufs=1) as wpool, \
             tc.tile_pool(name="stage", bufs=2) as stpool, \
             tc.tile_pool(name="work", bufs=3) as spool, \
             tc.tile_pool(name="state", bufs=3) as hpool, \
             tc.tile_pool(name="evp", bufs=2) as evpool, \
             tc.tile_pool(name="ps_g", bufs=2, space="PSUM") as ps_g, \
             tc.tile_pool(name="ps_big", bufs=2, space="PSUM") as ps_big, \
             tc.tile_pool(name="ps_fc", bufs=1, space="PSUM") as ps_fc:

            whh = wpool.tile([128, 64 * 128], FP8, tag="whh")
            nc.sync.dma_start(out=whh[:, :], in_=whhT[:, :])
            wih = wpool.tile([128, 64 * 128], BF16, tag="wih")
            nc.sync.dma_start(out=wih[:, :], in_=wihT[:, :])
            fcw_raw = stpool.tile([128, 4], BF16, tag="fcwraw")
            nc.sync.dma_start(out=fcw_raw[:, :], in_=fcwT[:, :])
            fcw_sb = wpool.tile([128, 4], BF16, tag="fcwf")
            nc.vector.tensor_copy(fcw_sb[:, :], fcw_raw[:, :])
            braw = stpool.tile([128, 16], F32, tag="braw")
            nc.sync.dma_start(out=braw[:, :], in_=br[:, :])
            b_sb = wpool.tile([128, 16], F32, tag="bf")
            nc.vector.tensor_copy(b_sb[:, :], braw[:, :])

            # xg rhs double buffer + y history double buffer
            R = wpool.tile([128, 2 * BLK], BF16, tag="Rbuf")
            ybl = wpool.tile([128, 2 * BLK], BF16, tag="ybl")
            nc.vector.memzero(R[:, :])

            pid = nc.partition_id()

            def wtile(wsb, k, m):
                return wsb[:, (k * 16 + m) * 128:(k * 16 + m) * 128 + 128]

            def rk(j, k):
                """R[j%2] k-chunk [128, TB*PB]."""
                off = (j % 2) * BLK + k * TB * PB
                return R[:, off:off + TB * PB]

            def yslot(j, k, t):
                """ybl[j%2] (k, t) h-slot [128, PB]."""
                off = (j % 2) * BLK + (k * TB + t) * PB
                return ybl[:, off:off + PB]

            # ---- xg GEMM unit: one (m, half-N) piece of a block ----
            HN = TB * PB // 2  # 512 = one PSUM bank

            def xg_unit(j, evb, u):
                m, h2 = u // 2, u % 2
                ps = ps_big.tile([128, HN], F32, tag="ps_gemm", name="psg")
                for k in range(4):
                    nc.tensor.matmul(
                        ps[:, :], lhsT=wtile(wih, k, m),
                        rhs=rk(j, k)[:, h2 * HN:(h2 + 1) * HN],
                        start=(k == 0), stop=(k == 3))
                dst = evb[:, m * TB * PB + h2 * HN:
                          m * TB * PB + (h2 + 1) * HN]
                if u % 2 == 0:
                    nc.vector.tensor_scalar_add(dst, ps[:, :],
                                                b_sb[:, m:m + 1])
                else:
                    # alternate the bias-add onto the Scalar engine to
                    # split this load between DVE and ACT
                    nc.scalar.activation(dst, ps[:, :], AF.Identity,
                                         bias=b_sb[:, m:m + 1])

            def new_evb():
                return evpool.tile([128, 16 * TB * PB], BF16, tag="evb",
                                   name="evb")

            # ---- one recurrence step, H-half pipelined ----
            # returns new c tiles (one per half)
            def step(j, t, evb, h_prev, c_cur, first):
                c_new = [None, None]
                for hf in range(2):
                    ms = hf * 8  # m-tile base of this half
                    if not first:
                        gp = ps_g.tile([128, 8 * PB], F32, tag=f"gp{hf}",
                                       name="gp")
                        for k in range(4):
                            for mi in range(8):
                                nc.tensor.matmul(
                                    gp[:, mi * PB:(mi + 1) * PB],
                                    lhsT=wtile(whh, k, ms + mi),
                                    rhs=h_prev[k],
                                    start=(k == 0), stop=(k == 3))
                        s_pre = spool.tile([128, 8 * PB], F32, tag=f"sp{hf}",
                                           name="spre")
                        xg_ap = evb[:, :].rearrange(
                            "p (m t b) -> p m t b", m=16, t=TB)[:, ms:ms + 8, t, :]
                        nc.vector.tensor_add(
                            s_pre[:, :].rearrange("p (m b) -> p m b", m=8),
                            gp[:, :].rearrange("p (m b) -> p m b", m=8), xg_ap)
                        src_sig = s_pre[:, :]
                        rr = False
                    else:
                        xg4 = evb[:, :].rearrange(
                            "p (m t b) -> p m t b", m=16, t=TB)
                        src_sig = xg4[:, ms:ms + 8, t, :]
                        rr = True
                    # one sigmoid for all 8 m-tiles; the g-gate rows carry an
                    # extra 2x in their weights so slot u = sig(2g) and
                    # tanh(g) = 2u - 1
                    s_sig = spool.tile([128, 8 * PB], BF16, tag=f"ss{hf}",
                                       name="ssig")
                    nc.scalar.activation(
                        s_sig[:, :].rearrange("p (m b) -> p m b", m=8)
                        if rr else s_sig[:, :],
                        src_sig, AF.Sigmoid, scale=INV_WSCL)
                    s_tg = spool.tile([128, 2 * PB], BF16, tag=f"st{hf}",
                                      name="stg")
                    nc.vector.tensor_scalar(
                        s_tg[:, :], s_sig[:, 6 * PB:], 2.0, -1.0,
                        mybir.AluOpType.mult, mybir.AluOpType.add)
                    tmp = spool.tile([128, 2 * PB], BF16, tag=f"tm{hf}",
                                     name="tmp")
                    nc.vector.tensor_mul(tmp[:, :], s_sig[:, :2 * PB],
                                         s_tg[:, :])
                    cn = hpool.tile([128, 2 * PB], BF16, tag=f"c{hf}", name="cn")
                    if not first:
                        nc.vector.tensor_mul(cn[:, :], s_sig[:, 2 * PB:4 * PB],
                                             c_cur[hf][:, :])
                        nc.vector.tensor_add(cn[:, :], cn[:, :], tmp[:, :])
                    else:
                        nc.vector.tensor_copy(cn[:, :], tmp[:, :])
                    s_tc = spool.tile([128, 2 * PB], BF16, tag=f"sc{hf}",
                                      name="stc")
                    nc.scalar.activation(s_tc[:, :], cn[:, :], AF.Tanh)
                    # h chunks 2hf, 2hf+1 -> ybl slots (strided 2-chunk view)
                    hdst = ybl[:, :].rearrange(
                        "p (s k t b) -> p s k t b", s=2, k=4, t=TB)[
                        :, j % 2, 2 * hf:2 * hf + 2, t, :]
                    nc.vector.tensor_mul(hdst, s_sig[:, 4 * PB:6 * PB]
                                         .rearrange("p (m b) -> p m b", m=2),
                                         s_tc[:, :]
                                         .rearrange("p (m b) -> p m b", m=2))
                    c_new[hf] = cn
                return c_new

            # ---- main loop over blocks; the xg GEMM of block j+1 is
            # interleaved one (m, half) unit per step of block j so the PE
            # has independent work while step activation chains resolve ----
            def fill_R(j):
                with tc.If(pid < 4) as cmp:
                    # layer-0 role: fill R[j%2] with x block j
                    nc.sync.dma_start(
                        out=R[:, (j % 2) * BLK:(j % 2 + 1) * BLK],
                        in_=xTd[:, :].rearrange(
                            "p (k t b) -> p k t b", k=4, t=NITER * TB)
                        [:, :, j * TB:(j + 1) * TB, :])
                with cmp.Else():
                    if j >= 2:
                        # layer-1 role: fill R[j%2] with partner y0 block j-2
                        nc.sync.dma_start(
                            out=R[:, (j % 2) * BLK:(j % 2 + 1) * BLK],
                            in_=cc_out[(j - 2) % 3, :, :BLK])

            c_cur = None
            hT0 = stpool.tile([128, 4 * PB], BF16, tag="hT0")
            fill_R(0)
            evb_cur = new_evb()
            for u in range(32):
                xg_unit(0, evb_cur, u)
            for j in range(NITER):
                if j == NITER - 2:
                    # snapshot layer-0 role's final h (end of data block 15)
                    # before iteration 17 overwrites that ybl parity slot
                    nc.vector.tensor_copy(
                        hT0[:, :].rearrange("p (k b) -> p k b", k=4),
                        ybl[:, :].rearrange("p (s k t b) -> p s k t b",
                                            s=2, k=4, t=TB)
                        [:, (NITER - 3) % 2, :, TB - 1, :])
                evb_next = None
                if j + 1 < NITER:
                    fill_R(j + 1)
                    evb_next = new_evb()
                for tt in range(TB):
                    tprog = j * TB + tt
                    if tprog == 0:
                        h_prev = None
                    elif tt == 0:
                        h_prev = [yslot(j - 1, k, TB - 1) for k in range(4)]
                    else:
                        h_prev = [yslot(j, k, tt - 1) for k in range(4)]
                    c_cur = step(j, tt, evb_cur, h_prev, c_cur, tprog == 0)
                    if evb_next is not None:
                        # next block's xg piece trails the step so the
                        # critical-path ops lead each engine's queue
                        xg_unit(j + 1, evb_next, tt)
                evb_cur = evb_next
                if j < NITER - 2:
                    nc.sync.dma_start(
                        out=cc_in[j % 3, :, :],
                        in_=ybl[:, (j % 2) * BLK:(j % 2 + 1) * BLK])
                    nc.gpsimd.collective_compute(
                        "AllGather", mybir.AluOpType.bypass,
                        ins=[cc_in[j % 3, :, :]],
                        outs=[cc_out[j % 3, :, :]],
                        replica_groups=RG)

            # ---- fc head: candidate final h: layer-0 role from the block-15
            # snapshot, layer-1 role from its last iteration; host selects ----
            hT1 = stpool.tile([128, 4 * PB], BF16, tag="hT1")
            nc.vector.tensor_copy(
                hT1[:, :].rearrange("p (k b) -> p k b", k=4),
                ybl[:, :].rearrange("p (s k t b) -> p s k t b",
                                    s=2, k=4, t=TB)
                [:, (NITER - 1) % 2, :, TB - 1, :])
            for li, hT in ((0, hT0), (1, hT1)):
                ps = ps_fc.tile([PB, 1], F32, tag="ps_fc", name="psfc")
                for k in range(4):
                    nc.tensor.matmul(ps[:, :], lhsT=hT[:, k * PB:(k + 1) * PB],
                                     rhs=fcw_sb[:, k:k + 1],
                                     start=(k == 0), stop=(k == 3))
                ov = spool.tile([PB, 1], F32, tag="ov", name="ov")
                nc.vector.tensor_scalar_add(ov[:, :], ps[:, :], 30.0)
                nc.sync.dma_start(out=out[li * PB:(li + 1) * PB, :],
                                  in_=ov[:, :])
    return nc


_cache = {}


def build_kernel():
    if "k" not in _cache:
        nc = bacc.Bacc("TRN2", target_bir_lowering=False, debug=False,
                       num_devices=NC)
        _build(nc)
        nc.compile()
        _cache["k"] = nc
    return _cache["k"]


def _wT_host(w, dtype):
    """w [G, 512] f32 (zero-padded cols if needed) -> [128, 64*128];
    block (k,m) = w[M_SRC[m]*128:+128, k*128:+128].T"""
    outw = np.empty((128, 64 * 128), dtype=dtype)
    for k in range(4):
        for m in range(16):
            blk = w[M_SRC[m] * 128:(M_SRC[m] + 1) * 128,
                    k * 128:(k + 1) * 128].T
            outw[:, (k * 16 + m) * 128:(k * 16 + m + 1) * 128] = blk.astype(dtype)
    return outw


def _prep_role(w_ih, w_hh, bb):
    """Per-role (layer) weight staging; w_ih padded to 512 cols. The
    g-gate rows (2H:3H) get an extra 2x so the single sigmoid computes
    u = sig(2g) and tanh(g) = 2u - 1 in a cheap fused scalar op."""
    kin = w_ih.shape[1]
    wi = np.zeros((G, 512), np.float32)
    wi[:, :kin] = w_ih.astype(np.float32) * WSCL
    wh = w_hh.astype(np.float32) * WSCL
    b = bb.astype(np.float32).reshape(G) * WSCL
    wi[2 * H:3 * H] *= 2.0
    wh = wh.copy()
    wh[2 * H:3 * H] *= 2.0
    b = b.copy()
    b[2 * H:3 * H] *= 2.0
    brr = np.stack([b[M_SRC[m] * 128:(M_SRC[m] + 1) * 128]
                    for m in range(16)], 1)
    return {
        "whhT": _wT_host(wh, FP8NP),
        "wihT": _wT_host(wi, BF16NP),
        "br": np.ascontiguousarray(brr),
    }


def run(inputs, **kw):
    nc = build_kernel()
    x = inputs["x"].astype(np.float32)
    fcw = inputs["fc_w"].astype(np.float32).reshape(H)
    fcm = np.ascontiguousarray(fcw.reshape(4, 128).T.astype(BF16NP))
    role0 = _prep_role(inputs["w_ih0"], inputs["w_hh0"], inputs["b0"])
    role1 = _prep_role(inputs["w_ih1"], inputs["w_hh1"], inputs["b1"])
    xz = np.zeros((128, 4 * NITER * TB * PB), BF16NP)
    in_maps = []
    for c in range(NC):
        role = role0 if c < 4 else role1
        m = dict(role)
        m["fcwT"] = fcm
        if c < 4:
            xs = x[c * PB:(c + 1) * PB]              # [PB, T, D]
            xt = np.zeros((128, 4, NITER * TB, PB), np.float32)
            xsw = xs.reshape(PB, T, 2, 128).transpose(3, 2, 1, 0)
            xt[:, :2, :T, :] = xsw
            m["xT"] = np.ascontiguousarray(
                xt.reshape(128, 4 * NITER * TB * PB)).astype(BF16NP)
        else:
            m["xT"] = xz
        in_maps.append(m)
    res = run_bass_kernel_spmd(nc, in_maps, core_ids=list(range(NC)), **kw)
    outp = np.zeros((2 * B, 1), np.float32)
    for c in range(NC):
        r = res.results[c]["out"]
        if c < 4:
            outp[c * PB:(c + 1) * PB] = r[:PB]       # layer-0 hn rows
        else:
            cc = c - 4
            outp[B + cc * PB:B + (cc + 1) * PB] = r[PB:]  # layer-1 rows
    return outp, res


def kernel(**inputs):
    outp, _ = run(inputs)
    return outp


# revision 24
# speedup vs baseline: 1.0382x; 1.0223x over previous
"""Trainium2 Bass kernel for a 2-layer LSTM + fc head.

Strategy (v2): LAYER-PIPELINED across core pairs. Cores 0-3 run layer 0
(32 batch rows each), cores 4-7 run layer 1 for the same rows, lagged
one TB-step block. y0 history blocks travel c -> c+4 once per block via
a pair AllGather (rank-0 slot delivery), hidden under the one-block lag.
This halves the PE instruction count per core per step: the recurrence
is issue-bound (~60ns per LDWEIGHTS+MATMUL pair regardless of free dim
16/32/64), so per-step cost scales with instruction count, not batch.

Recurrent weights are fp8(e3m4) scaled by 2^WSH (FWL loads 4B/cycle);
input-projection weights/biases carry the same scale; the gate
activations undo it via their input-scale operand. h stays bf16.

Both roles execute one shared SPMD program; the only divergent pieces
(x-vs-received fill of the xg rhs buffer R) sit in tc.If(pid<4) blocks.
Layer-1 cores process a garbage block 0 (R zeroed -> gates from bias
only); the resulting ~1e-2 state perturbation decays via the forget
gate over 512 real steps.

Step internals split the gates into two H-halves so the activation
chain of half A overlaps the PE work of half B and of the next step.
m-tile order: [i0 i1 f0 f1 o0 o1 g0 g1 | i2 i3 f2 f3 o2 o3 g2 g3] so
each half's sigmoid (i,f,o) and tanh (g) slabs are contiguous.
"""

import numpy as np
import ml_dtypes
import concourse.bass as bass
import concourse.bacc as bacc
import concourse.mybir as mybir
from concourse.bass_utils import run_bass_kernel_spmd
from concourse.tile import TileContext

F32 = mybir.dt.float32
BF16 = mybir.dt.bfloat16
FP8 = mybir.dt.float8e3
AF = mybir.ActivationFunctionType
BF16NP = ml_dtypes.bfloat16
FP8NP = ml_dtypes.float8_e3m4

B, T, D, H = 128, 512, 256, 512
G = 4 * H
NC = 8
PB = 32            # batch rows per core pair
TB = 32            # timesteps per block
NITER = T // TB + 2  # 18: layer-1 role lags two blocks (hides AllGather)
WSH = 7
WSCL = float(2 ** WSH)
INV_WSCL = 1.0 / WSCL

# m-tile order (source 128-row blocks, PyTorch gate order i=0-3 f=4-7
# g=8-11 o=12-15): per H-half [i i f f o o g g]
M_SRC = [0, 1, 4, 5, 12, 13, 8, 9, 2, 3, 6, 7, 14, 15, 10, 11]
RG = [[0, 4], [1, 5], [2, 6], [3, 7]]


def _build(nc):
    whhT = nc.declare_dram_parameter("whhT", [128, 64 * 128], FP8, isOutput=False)
    wihT = nc.declare_dram_parameter("wihT", [128, 64 * 128], BF16, isOutput=False)
    br = nc.declare_dram_parameter("br", [128, 16], F32, isOutput=False)
    fcwT = nc.declare_dram_parameter("fcwT", [128, 4], BF16, isOutput=False)
    # x, host-transposed: [128, (kd, t, b)] kd=4 (zero-padded for L0)
    xTd = nc.declare_dram_parameter("xT", [128, 4 * NITER * TB * PB], BF16,
                                    isOutput=False)
    out = nc.declare_dram_parameter("out", [2 * PB, 1], F32, isOutput=True)

    BLK = 4 * TB * PB  # cols per (k,t,b) block
    cc_in = nc.dram_tensor("cc_in", [3, 128, BLK], BF16, kind="Internal")
    cc_out = nc.dram_tensor("cc_out", [3, 128, 2 * BLK], BF16, kind="Internal")

    with TileContext(nc) as tc:
        with tc.tile_pool(name="wts", bufs=1) as wpool, \
             tc.tile_pool(name="stage", bufs=2) as stpool, \
             tc.tile_pool(name="work", bufs=3) as spool, \
             tc.tile_pool(name="state", bufs=3) as hpool, \
             tc.tile_pool(name="evp", bufs=2) as evpool, \
             tc.tile_pool(name="ps_g", bufs=2, space="PSUM") as ps_g, \
             tc.tile_pool(name="ps_big", bufs=2, space="PSUM") as ps_big, \
             tc.tile_pool(name="ps_fc", bufs=1, space="PSUM") as ps_fc:

            whh = wpool.tile([128, 64 * 128], FP8, tag="whh")
            nc.sync.dma_start(out=whh[:, :], in_=whhT[:, :])
            wih = wpool.tile([128, 64 * 128], BF16, tag="wih")
            nc.sync.dma_start(out=wih[:, :], in_=wihT[:, :])
            fcw_raw = stpool.tile([128, 4], BF16, tag="fcwraw")
            nc.sync.dma_start(out=fcw_raw[:, :], in_=fcwT[:, :])
            fcw_sb = wpool.tile([128, 4], BF16, tag="fcwf")
            nc.vector.tensor_copy(fcw_sb[:, :], fcw_raw[:, :])
            braw = stpool.tile([128, 16], F32, tag="braw")
            nc.sync.dma_start(out=braw[:, :], in_=br[:, :])
            b_sb = wpool.tile([128, 16], F32, tag="bf")
            nc.vector.tensor_copy(b_sb[:, :], braw[:, :])

            # xg rhs double buffer + y history double buffer
            R = wpool.tile([128, 2 * BLK], BF16, tag="Rbuf")
            ybl = wpool.tile([128, 2 * BLK], BF16, tag="ybl")
            nc.vector.memzero(R[:, :])

            pid = nc.partition_id()

            def wtile(wsb, k, m):
                return wsb[:, (k * 16 + m) * 128:(k * 16 + m) * 128 + 128]

            def rk(j, k):
                """R[j%2] k-chunk [128, TB*PB]."""
                off = (j % 2) * BLK + k * TB * PB
                return R[:, off:off + TB * PB]

            def yslot(j, k, t):
                """ybl[j%2] (k, t) h-slot [128, PB]."""
                off = (j % 2) * BLK + (k * TB + t) * PB
                return ybl[:, off:off + PB]

            # ---- xg GEMM unit: one (m, half-N) piece of a block ----
            HN = TB * PB // 2  # 512 = one PSUM bank

            def xg_unit(j, evb, u):
                m, h2 = u // 2, u % 2
                ps = ps_big.tile([128, HN], F32, tag="ps_gemm", name="psg")
                for k in range(4):
                    nc.tensor.matmul(
                        ps[:, :], lhsT=wtile(wih, k, m),
                        rhs=rk(j, k)[:, h2 * HN:(h2 + 1) * HN],
                        start=(k == 0), stop=(k == 3))
                dst = evb[:, m * TB * PB + h2 * HN:
                          m * TB * PB + (h2 + 1) * HN]
                if u % 2 == 0:
                    nc.vector.tensor_scalar_add(dst, ps[:, :],
                                                b_sb[:, m:m + 1])
                else:
                    # alternate the bias-add onto the Scalar engine to
                    # split this load between DVE and ACT
                    nc.scalar.activation(dst, ps[:, :], AF.Identity,
                                         bias=b_sb[:, m:m + 1])

            def new_evb():
                return evpool.tile([128, 16 * TB * PB], BF16, tag="evb",
                                   name="evb")

            # ---- one recurrence step, H-half pipelined ----
            # returns new c tiles (one per half)
            def step(j, t, evb, h_prev, c_cur, first):
                c_new = [None, None]
                for hf in range(2):
                    ms = hf * 8  # m-tile base of this half
                    if not first:
                        gp = ps_g.tile([128, 8 * PB], F32, tag=f"gp{hf}",
                                       name="gp")
                        for k in range(4):
                            for mi in range(8):
                                nc.tensor.matmul(
                                    gp[:, mi * PB:(mi + 1) * PB],
                                    lhsT=wtile(whh, k, ms + mi),
                                    rhs=h_prev[k],
                                    start=(k == 0), stop=(k == 3))
                        s_pre = spool.tile([128, 8 * PB], F32, tag=f"sp{hf}",
                                           name="spre")
                        xg_ap = evb[:, :].rearrange(
                            "p (m t b) -> p m t b", m=16, t=TB)[:, ms:ms + 8, t, :]
                        nc.vector.tensor_add(
                            s_pre[:, :].rearrange("p (m b) -> p m b", m=8),
                            gp[:, :].rearrange("p (m b) -> p m b", m=8), xg_ap)
                        src_sig = s_pre[:, :]
                        rr = False
                    else:
                        xg4 = evb[:, :].rearrange(
                            "p (m t b) -> p m t b", m=16, t=TB)
                        src_sig = xg4[:, ms:ms + 8, t, :]
                        rr = True
                    # one sigmoid for all 8 m-tiles; the g-gate rows carry an
                    # extra 2x in their weights so slot u = sig(2g) and
                    # tanh(g) = 2u - 1
                    s_sig = spool.tile([128, 8 * PB], BF16, tag=f"ss{hf}",
                                       name="ssig")
                    nc.scalar.activation(
                        s_sig[:, :].rearrange("p (m b) -> p m b", m=8)
                        if rr else s_sig[:, :],
                        src_sig, AF.Sigmoid, scale=INV_WSCL)
                    s_tg = spool.tile([128, 2 * PB], BF16, tag=f"st{hf}",
                                      name="stg")
                    nc.vector.tensor_scalar(
                        s_tg[:, :], s_sig[:, 6 * PB:], 2.0, -1.0,
                        mybir.AluOpType.mult, mybir.AluOpType.add)
                    tmp = spool.tile([128, 2 * PB], BF16, tag=f"tm{hf}",
                                     name="tmp")
                    nc.vector.tensor_mul(tmp[:, :], s_sig[:, :2 * PB],
                                         s_tg[:, :])
                    cn = hpool.tile([128, 2 * PB], BF16, tag=f"c{hf}", name="cn")
                    if not first:
                        nc.vector.tensor_mul(cn[:, :], s_sig[:, 2 * PB:4 * PB],
                                             c_cur[hf][:, :])
                        nc.vector.tensor_add(cn[:, :], cn[:, :], tmp[:, :])
                    else:
                        nc.vector.tensor_copy(cn[:, :], tmp[:, :])
                    s_tc = spool.tile([128, 2 * PB], BF16, tag=f"sc{hf}",
                                      name="stc")
                    nc.scalar.activation(s_tc[:, :], cn[:, :], AF.Tanh)
                    # h chunks 2hf, 2hf+1 -> ybl slots (strided 2-chunk view)
                    hdst = ybl[:, :].rearrange(
                        "p (s k t b) -> p s k t b", s=2, k=4, t=TB)[
                        :, j % 2, 2 * hf:2 * hf + 2, t, :]
                    nc.vector.tensor_mul(hdst, s_sig[:, 4 * PB:6 * PB]
                                         .rearrange("p (m b) -> p m b", m=2),
                                         s_tc[:, :]
                                         .rearrange("p (m b) -> p m b", m=2))
                    c_new[hf] = cn
                return c_new

            # ---- main loop over blocks; the xg GEMM of block j+1 is
            # interleaved one (m, half) unit per step of block j so the PE
            # has independent work while step activation chains resolve ----
            def fill_R(j):
                with tc.If(pid < 4) as cmp:
                    # layer-0 role: fill R[j%2] with x block j
                    nc.sync.dma_start(
                        out=R[:, (j % 2) * BLK:(j % 2 + 1) * BLK],
                        in_=xTd[:, :].rearrange(
                            "p (k t b) -> p k t b", k=4, t=NITER * TB)
                        [:, :, j * TB:(j + 1) * TB, :])
                with cmp.Else():
                    if j >= 2:
                        # layer-1 role: fill R[j%2] with partner y0 block j-2
                        nc.sync.dma_start(
                            out=R[:, (j % 2) * BLK:(j % 2 + 1) * BLK],
                            in_=cc_out[(j - 2) % 3, :, :BLK])

            c_cur = None
            hT0 = stpool.tile([128, 4 * PB], BF16, tag="hT0")
            for j in range(NITER):
                if j == NITER - 2:
                    # snapshot layer-0 role's final h (end of data block 15)
                    # before iteration 17 overwrites that ybl parity slot
                    nc.vector.tensor_copy(
                        hT0[:, :].rearrange("p (k b) -> p k b", k=4),
                        ybl[:, :].rearrange("p (s k t b) -> p s k t b",
                                            s=2, k=4, t=TB)
                        [:, (NITER - 3) % 2, :, TB - 1, :])
                fill_R(j)
                evb_cur = new_evb()
                # block's xg as one burst before its steps; spreading it
                # per-step measured slower (it delays each step's critical
                # MMs in the in-order PE queue)
                for u in range(32):
                    xg_unit(j, evb_cur, u)
                for tt in range(TB):
                    tprog = j * TB + tt
                    if tprog == 0:
                        h_prev = None
                    elif tt == 0:
                        h_prev = [yslot(j - 1, k, TB - 1) for k in range(4)]
                    else:
                        h_prev = [yslot(j, k, tt - 1) for k in range(4)]
                    c_cur = step(j, tt, evb_cur, h_prev, c_cur, tprog == 0)
                if j < NITER - 2:
                    nc.sync.dma_start(
                        out=cc_in[j % 3, :, :],
                        in_=ybl[:, (j % 2) * BLK:(j % 2 + 1) * BLK])
                    nc.gpsimd.collective_compute(
                        "AllGather", mybir.AluOpType.bypass,
                        ins=[cc_in[j % 3, :, :]],
                        outs=[cc_out[j % 3, :, :]],
                        replica_groups=RG)

            # ---- fc head: candidate final h: layer-0 role from the block-15
            # snapshot, layer-1 role from its last iteration; host selects ----
            hT1 = stpool.tile([128, 4 * PB], BF16, tag="hT1")
            nc.vector.tensor_copy(
                hT1[:, :].rearrange("p (k b) -> p k b", k=4),
                ybl[:, :].rearrange("p (s k t b) -> p s k t b",
                                    s=2, k=4, t=TB)
                [:, (NITER - 1) % 2, :, TB - 1, :])
            for li, hT in ((0, hT0), (1, hT1)):
                ps = ps_fc.tile([PB, 1], F32, tag="ps_fc", name="psfc")
                for k in range(4):
                    nc.tensor.matmul(ps[:, :], lhsT=hT[:, k * PB:(k + 1) * PB],
                                     rhs=fcw_sb[:, k:k + 1],
                                     start=(k == 0), stop=(k == 3))
                ov = spool.tile([PB, 1], F32, tag="ov", name="ov")
                nc.vector.tensor_scalar_add(ov[:, :], ps[:, :], 30.0)
                nc.sync.dma_start(out=out[li * PB:(li + 1) * PB, :],
                                  in_=ov[:, :])
    return nc


_cache = {}


def build_kernel():
    if "k" not in _cache:
        nc = bacc.Bacc("TRN2", target_bir_lowering=False, debug=False,
                       num_devices=NC)
        _build(nc)
        nc.compile()
        _cache["k"] = nc
    return _cache["k"]


def _wT_host(w, dtype):
    """w [G, 512] f32 (zero-padded cols if needed) -> [128, 64*128];
    block (k,m) = w[M_SRC[m]*128:+128, k*128:+128].T"""
    outw = np.empty((128, 64 * 128), dtype=dtype)
    for k in range(4):
        for m in range(16):
            blk = w[M_SRC[m] * 128:(M_SRC[m] + 1) * 128,
                    k * 128:(k + 1) * 128].T
            outw[:, (k * 16 + m) * 128:(k * 16 + m + 1) * 128] = blk.astype(dtype)
    return outw


def _prep_role(w_ih, w_hh, bb):
    """Per-role (layer) weight staging; w_ih padded to 512 cols. The
    g-gate rows (2H:3H) get an extra 2x so the single sigmoid computes
    u = sig(2g) and tanh(g) = 2u - 1 in a cheap fused scalar op."""
    kin = w_ih.shape[1]
    wi = np.zeros((G, 512), np.float32)
    wi[:, :kin] = w_ih.astype(np.float32) * WSCL
    wh = w_hh.astype(np.float32) * WSCL
    b = bb.astype(np.float32).reshape(G) * WSCL
    wi[2 * H:3 * H] *= 2.0
    wh = wh.copy()
    wh[2 * H:3 * H] *= 2.0
    b = b.copy()
    b[2 * H:3 * H] *= 2.0
    brr = np.stack([b[M_SRC[m] * 128:(M_SRC[m] + 1) * 128]
                    for m in range(16)], 1)
    return {
        "whhT": _wT_host(wh, FP8NP),
        "wihT": _wT_host(wi, BF16NP),
        "br": np.ascontiguousarray(brr),
    }


def run(inputs, **kw):
    nc = build_kernel()
    x = inputs["x"].astype(np.float32)
    fcw = inputs["fc_w"].astype(np.float32).reshape(H)
    fcm = np.ascontiguousarray(fcw.reshape(4, 128).T.astype(BF16NP))
    role0 = _prep_role(inputs["w_ih0"], inputs["w_hh0"], inputs["b0"])
    role1 = _prep_role(inputs["w_ih1"], inputs["w_hh1"], inputs["b1"])
    xz = np.zeros((128, 4 * NITER * TB * PB), BF16NP)
    in_maps = []
    for c in range(NC):
        role = role0 if c < 4 else role1
        m = dict(role)
        m["fcwT"] = fcm
        if c < 4:
            xs = x[c * PB:(c + 1) * PB]              # [PB, T, D]
            xt = np.zeros((128, 4, NITER * TB, PB), np.float32)
            xsw = xs.reshape(PB, T, 2, 128).transpose(3, 2, 1, 0)
            xt[:, :2, :T, :] = xsw
            m["xT"] = np.ascontiguousarray(
                xt.reshape(128, 4 * NITER * TB * PB)).astype(BF16NP)
        else:
            m["xT"] = xz
        in_maps.append(m)
    res = run_bass_kernel_spmd(nc, in_maps, core_ids=list(range(NC)), **kw)
    outp = np.zeros((2 * B, 1), np.float32)
    for c in range(NC):
        r = res.results[c]["out"]
        if c < 4:
            outp[c * PB:(c + 1) * PB] = r[:PB]       # layer-0 hn rows
        else:
            cc = c - 4
            outp[B + cc * PB:B + (cc + 1) * PB] = r[PB:]  # layer-1 rows
    return outp, res


def kernel(**inputs):
    outp, _ = run(inputs)
    return outp


# revision 26
# speedup vs baseline: 1.0866x; 1.0466x over previous
"""Trainium2 Bass kernel for a 2-layer LSTM + fc head.

Strategy (v2): LAYER-PIPELINED across core pairs. Cores 0-3 run layer 0
(32 batch rows each), cores 4-7 run layer 1 for the same rows, lagged
one TB-step block. y0 history blocks travel c -> c+4 once per block via
a pair AllGather (rank-0 slot delivery), hidden under the one-block lag.
This halves the PE instruction count per core per step: the recurrence
is issue-bound (~60ns per LDWEIGHTS+MATMUL pair regardless of free dim
16/32/64), so per-step cost scales with instruction count, not batch.

Recurrent weights are fp8(e3m4) scaled by 2^WSH (FWL loads 4B/cycle);
input-projection weights/biases carry the same scale; the gate
activations undo it via their input-scale operand. h stays bf16.

Both roles execute one shared SPMD program; the only divergent pieces
(x-vs-received fill of the xg rhs buffer R) sit in tc.If(pid<4) blocks.
Layer-1 cores process a garbage block 0 (R zeroed -> gates from bias
only); the resulting ~1e-2 state perturbation decays via the forget
gate over 512 real steps.

Step internals split the gates into two H-halves so the activation
chain of half A overlaps the PE work of half B and of the next step.
m-tile order: [i0 i1 f0 f1 o0 o1 g0 g1 | i2 i3 f2 f3 o2 o3 g2 g3] so
each half's sigmoid (i,f,o) and tanh (g) slabs are contiguous.
"""

import numpy as np
import ml_dtypes
import concourse.bass as bass
import concourse.bacc as bacc
import concourse.mybir as mybir
from concourse.bass_utils import run_bass_kernel_spmd
from concourse.tile import TileContext

F32 = mybir.dt.float32
BF16 = mybir.dt.bfloat16
FP8 = mybir.dt.float8e3
AF = mybir.ActivationFunctionType
BF16NP = ml_dtypes.bfloat16
FP8NP = ml_dtypes.float8_e3m4

B, T, D, H = 128, 512, 256, 512
G = 4 * H
NC = 8
PB = 32            # batch rows per core pair
TB = 32            # timesteps per block
NITER = T // TB + 2  # 18: layer-1 role lags two blocks (hides AllGather)
WSH = 7
WSCL = float(2 ** WSH)
INV_WSCL = 1.0 / WSCL

# m-tile order (source 128-row blocks, PyTorch gate order i=0-3 f=4-7
# g=8-11 o=12-15): per H-half [i i f f o o g g]
M_SRC = [0, 1, 4, 5, 12, 13, 8, 9, 2, 3, 6, 7, 14, 15, 10, 11]
RG = [[0, 4], [1, 5], [2, 6], [3, 7]]


def _build(nc):
    whhT = nc.declare_dram_parameter("whhT", [128, 64 * 128], FP8, isOutput=False)
    wihT = nc.declare_dram_parameter("wihT", [128, 64 * 128], BF16, isOutput=False)
    br = nc.declare_dram_parameter("br", [128, 16], F32, isOutput=False)
    fcwT = nc.declare_dram_parameter("fcwT", [128, 4], BF16, isOutput=False)
    # x, host-transposed: [128, (kd, t, b)] kd=4 (zero-padded for L0)
    xTd = nc.declare_dram_parameter("xT", [128, 4 * NITER * TB * PB], BF16,
                                    isOutput=False)
    out = nc.declare_dram_parameter("out", [2 * PB, 1], F32, isOutput=True)

    BLK = 4 * TB * PB  # cols per (k,t,b) block
    cc_in = nc.dram_tensor("cc_in", [3, 128, BLK], BF16, kind="Internal")
    cc_out = nc.dram_tensor("cc_out", [3, 128, 2 * BLK], BF16, kind="Internal")

    with TileContext(nc) as tc:
        with tc.tile_pool(name="wts", bufs=1) as wpool, \
             tc.tile_pool(name="stage", bufs=2) as stpool, \
             tc.tile_pool(name="work", bufs=3) as spool, \
             tc.tile_pool(name="state", bufs=3) as hpool, \
             tc.tile_pool(name="evp", bufs=2) as evpool, \
             tc.tile_pool(name="ps_g", bufs=2, space="PSUM") as ps_g, \
             tc.tile_pool(name="ps_big", bufs=2, space="PSUM") as ps_big, \
             tc.tile_pool(name="ps_fc", bufs=1, space="PSUM") as ps_fc:

            whh = wpool.tile([128, 64 * 128], FP8, tag="whh")
            nc.sync.dma_start(out=whh[:, :], in_=whhT[:, :])
            wih = wpool.tile([128, 64 * 128], BF16, tag="wih")
            nc.sync.dma_start(out=wih[:, :], in_=wihT[:, :])
            fcw_raw = stpool.tile([128, 4], BF16, tag="fcwraw")
            nc.sync.dma_start(out=fcw_raw[:, :], in_=fcwT[:, :])
            fcw_sb = wpool.tile([128, 4], BF16, tag="fcwf")
            nc.vector.tensor_copy(fcw_sb[:, :], fcw_raw[:, :])
            braw = stpool.tile([128, 16], F32, tag="braw")
            nc.sync.dma_start(out=braw[:, :], in_=br[:, :])
            b_sb = wpool.tile([128, 16], F32, tag="bf")
            nc.vector.tensor_copy(b_sb[:, :], braw[:, :])

            # xg rhs double buffer + y history double buffer
            R = wpool.tile([128, 2 * BLK], BF16, tag="Rbuf")
            ybl = wpool.tile([128, 2 * BLK], BF16, tag="ybl")
            nc.vector.memzero(R[:, :])

            pid = nc.partition_id()

            def wtile(wsb, k, m):
                return wsb[:, (k * 16 + m) * 128:(k * 16 + m) * 128 + 128]

            def rk(j, k):
                """R[j%2] k-chunk [128, TB*PB]."""
                off = (j % 2) * BLK + k * TB * PB
                return R[:, off:off + TB * PB]

            def yslot(j, k, t):
                """ybl[j%2] (k, t) h-slot [128, PB]."""
                off = (j % 2) * BLK + (k * TB + t) * PB
                return ybl[:, off:off + PB]

            # ---- xg GEMM unit: one (m, half-N) piece of a block ----
            HN = TB * PB // 2  # 512 = one PSUM bank

            def xg_unit(j, evb, u):
                m, h2 = u // 2, u % 2
                ps = ps_big.tile([128, HN], F32, tag="ps_gemm", name="psg")
                for k in range(4):
                    nc.tensor.matmul(
                        ps[:, :], lhsT=wtile(wih, k, m),
                        rhs=rk(j, k)[:, h2 * HN:(h2 + 1) * HN],
                        start=(k == 0), stop=(k == 3))
                dst = evb[:, m * TB * PB + h2 * HN:
                          m * TB * PB + (h2 + 1) * HN]
                if u % 2 == 0:
                    nc.vector.tensor_scalar_add(dst, ps[:, :],
                                                b_sb[:, m:m + 1])
                else:
                    # alternate the bias-add onto the Scalar engine to
                    # split this load between DVE and ACT
                    nc.scalar.activation(dst, ps[:, :], AF.Identity,
                                         bias=b_sb[:, m:m + 1])

            def new_evb():
                return evpool.tile([128, 16 * TB * PB], BF16, tag="evb",
                                   name="evb")

            # ---- pre-copy the xg slice of step t into PSUM: the step's
            # matmuls then accumulate onto it (start=False; DVE writes set
            # has_written), deleting the gate add from the critical chain.
            # Emitted one step ahead so the copy leads the DVE queue. ----
            def precopy(evb, t):
                gps = []
                for hf in range(2):
                    gp = ps_g.tile([128, 8 * PB], F32, tag=f"gp{hf}",
                                   name="gp")
                    xg_ap = evb[:, :].rearrange(
                        "p (m t b) -> p m t b", m=16, t=TB)[
                        :, hf * 8:hf * 8 + 8, t, :]
                    nc.vector.tensor_copy(
                        gp[:, :].rearrange("p (m b) -> p m b", m=8), xg_ap)
                    gps.append(gp)
                return gps

            # ---- one recurrence step, H-half pipelined ----
            # returns new c tiles (one per half)
            def step(j, t, evb, h_prev, c_cur, first, gps):
                c_new = [None, None]
                for hf in range(2):
                    ms = hf * 8  # m-tile base of this half
                    if not first:
                        gp = gps[hf]
                        for k in range(4):
                            for mi in range(8):
                                nc.tensor.matmul(
                                    gp[:, mi * PB:(mi + 1) * PB],
                                    lhsT=wtile(whh, k, ms + mi),
                                    rhs=h_prev[k],
                                    start=False, stop=(k == 3),
                                    skip_group_check=True)
                        src_sig = gp[:, :]
                        rr = False
                    else:
                        xg4 = evb[:, :].rearrange(
                            "p (m t b) -> p m t b", m=16, t=TB)
                        src_sig = xg4[:, ms:ms + 8, t, :]
                        rr = True
                    # one sigmoid for all 8 m-tiles; the g-gate rows carry an
                    # extra 2x in their weights so slot u = sig(2g) and
                    # tanh(g) = 2u - 1
                    s_sig = spool.tile([128, 8 * PB], BF16, tag=f"ss{hf}",
                                       name="ssig")
                    nc.scalar.activation(
                        s_sig[:, :].rearrange("p (m b) -> p m b", m=8)
                        if rr else s_sig[:, :],
                        src_sig, AF.Sigmoid, scale=INV_WSCL)
                    s_tg = spool.tile([128, 2 * PB], BF16, tag=f"st{hf}",
                                      name="stg")
                    nc.vector.tensor_scalar(
                        s_tg[:, :], s_sig[:, 6 * PB:], 2.0, -1.0,
                        mybir.AluOpType.mult, mybir.AluOpType.add)
                    tmp = spool.tile([128, 2 * PB], BF16, tag=f"tm{hf}",
                                     name="tmp")
                    nc.vector.tensor_mul(tmp[:, :], s_sig[:, :2 * PB],
                                         s_tg[:, :])
                    cn = hpool.tile([128, 2 * PB], BF16, tag=f"c{hf}", name="cn")
                    if not first:
                        nc.vector.tensor_mul(cn[:, :], s_sig[:, 2 * PB:4 * PB],
                                             c_cur[hf][:, :])
                        nc.vector.tensor_add(cn[:, :], cn[:, :], tmp[:, :])
                    else:
                        nc.vector.tensor_copy(cn[:, :], tmp[:, :])
                    s_tc = spool.tile([128, 2 * PB], BF16, tag=f"sc{hf}",
                                      name="stc")
                    nc.scalar.activation(s_tc[:, :], cn[:, :], AF.Tanh)
                    # h chunks 2hf, 2hf+1 -> ybl slots (strided 2-chunk view)
                    hdst = ybl[:, :].rearrange(
                        "p (s k t b) -> p s k t b", s=2, k=4, t=TB)[
                        :, j % 2, 2 * hf:2 * hf + 2, t, :]
                    nc.vector.tensor_mul(hdst, s_sig[:, 4 * PB:6 * PB]
                                         .rearrange("p (m b) -> p m b", m=2),
                                         s_tc[:, :]
                                         .rearrange("p (m b) -> p m b", m=2))
                    c_new[hf] = cn
                return c_new

            # ---- main loop over blocks; the xg GEMM of block j+1 is
            # interleaved one (m, half) unit per step of block j so the PE
            # has independent work while step activation chains resolve ----
            def fill_R(j):
                with tc.If(pid < 4) as cmp:
                    # layer-0 role: fill R[j%2] with x block j
                    nc.sync.dma_start(
                        out=R[:, (j % 2) * BLK:(j % 2 + 1) * BLK],
                        in_=xTd[:, :].rearrange(
                            "p (k t b) -> p k t b", k=4, t=NITER * TB)
                        [:, :, j * TB:(j + 1) * TB, :])
                with cmp.Else():
                    if j >= 2:
                        # layer-1 role: fill R[j%2] with partner y0 block j-2
                        nc.sync.dma_start(
                            out=R[:, (j % 2) * BLK:(j % 2 + 1) * BLK],
                            in_=cc_out[(j - 2) % 3, :, :BLK])

            c_cur = None
            hT0 = stpool.tile([128, 4 * PB], BF16, tag="hT0")
            for j in range(NITER):
                if j == NITER - 2:
                    # snapshot layer-0 role's final h (end of data block 15)
                    # before iteration 17 overwrites that ybl parity slot
                    nc.vector.tensor_copy(
                        hT0[:, :].rearrange("p (k b) -> p k b", k=4),
                        ybl[:, :].rearrange("p (s k t b) -> p s k t b",
                                            s=2, k=4, t=TB)
                        [:, (NITER - 3) % 2, :, TB - 1, :])
                fill_R(j)
                evb_cur = new_evb()
                # block's xg as one burst before its steps; spreading it
                # per-step measured slower (it delays each step's critical
                # MMs in the in-order PE queue)
                for u in range(32):
                    xg_unit(j, evb_cur, u)
                gps = precopy(evb_cur, 0) if j > 0 else None
                for tt in range(TB):
                    gps_next = precopy(evb_cur, tt + 1) if tt + 1 < TB \
                        else None
                    tprog = j * TB + tt
                    if tprog == 0:
                        h_prev = None
                    elif tt == 0:
                        h_prev = [yslot(j - 1, k, TB - 1) for k in range(4)]
                    else:
                        h_prev = [yslot(j, k, tt - 1) for k in range(4)]
                    c_cur = step(j, tt, evb_cur, h_prev, c_cur, tprog == 0,
                                 gps)
                    gps = gps_next
                if j < NITER - 2:
                    nc.sync.dma_start(
                        out=cc_in[j % 3, :, :],
                        in_=ybl[:, (j % 2) * BLK:(j % 2 + 1) * BLK])
                    nc.gpsimd.collective_compute(
                        "AllGather", mybir.AluOpType.bypass,
                        ins=[cc_in[j % 3, :, :]],
                        outs=[cc_out[j % 3, :, :]],
                        replica_groups=RG)

            # ---- fc head: candidate final h: layer-0 role from the block-15
            # snapshot, layer-1 role from its last iteration; host selects ----
            hT1 = stpool.tile([128, 4 * PB], BF16, tag="hT1")
            nc.vector.tensor_copy(
                hT1[:, :].rearrange("p (k b) -> p k b", k=4),
                ybl[:, :].rearrange("p (s k t b) -> p s k t b",
                                    s=2, k=4, t=TB)
                [:, (NITER - 1) % 2, :, TB - 1, :])
            for li, hT in ((0, hT0), (1, hT1)):
                ps = ps_fc.tile([PB, 1], F32, tag="ps_fc", name="psfc")
                for k in range(4):
                    nc.tensor.matmul(ps[:, :], lhsT=hT[:, k * PB:(k + 1) * PB],
                                     rhs=fcw_sb[:, k:k + 1],
                                     start=(k == 0), stop=(k == 3))
                ov = spool.tile([PB, 1], F32, tag="ov", name="ov")
                nc.vector.tensor_scalar_add(ov[:, :], ps[:, :], 30.0)
                nc.sync.dma_start(out=out[li * PB:(li + 1) * PB, :],
                                  in_=ov[:, :])
    return nc


_cache = {}


def build_kernel():
    if "k" not in _cache:
        nc = bacc.Bacc("TRN2", target_bir_lowering=False, debug=False,
                       num_devices=NC)
        _build(nc)
        nc.compile()
        _cache["k"] = nc
    return _cache["k"]


def _wT_host(w, dtype):
    """w [G, 512] f32 (zero-padded cols if needed) -> [128, 64*128];
    block (k,m) = w[M_SRC[m]*128:+128, k*128:+128].T"""
    outw = np.empty((128, 64 * 128), dtype=dtype)
    for k in range(4):
        for m in range(16):
            blk = w[M_SRC[m] * 128:(M_SRC[m] + 1) * 128,
                    k * 128:(k + 1) * 128].T
            outw[:, (k * 16 + m) * 128:(k * 16 + m + 1) * 128] = blk.astype(dtype)
    return outw


def _prep_role(w_ih, w_hh, bb):
    """Per-role (layer) weight staging; w_ih padded to 512 cols. The
    g-gate rows (2H:3H) get an extra 2x so the single sigmoid computes
    u = sig(2g) and tanh(g) = 2u - 1 in a cheap fused scalar op."""
    kin = w_ih.shape[1]
    wi = np.zeros((G, 512), np.float32)
    wi[:, :kin] = w_ih.astype(np.float32) * WSCL
    wh = w_hh.astype(np.float32) * WSCL
    b = bb.astype(np.float32).reshape(G) * WSCL
    wi[2 * H:3 * H] *= 2.0
    wh = wh.copy()
    wh[2 * H:3 * H] *= 2.0
    b = b.copy()
    b[2 * H:3 * H] *= 2.0
    brr = np.stack([b[M_SRC[m] * 128:(M_SRC[m] + 1) * 128]
                    for m in range(16)], 1)
    return {
        "whhT": _wT_host(wh, FP8NP),
        "wihT": _wT_host(wi, BF16NP),
        "br": np.ascontiguousarray(brr),
    }


def run(inputs, **kw):
    nc = build_kernel()
    x = inputs["x"].astype(np.float32)
    fcw = inputs["fc_w"].astype(np.float32).reshape(H)
    fcm = np.ascontiguousarray(fcw.reshape(4, 128).T.astype(BF16NP))
    role0 = _prep_role(inputs["w_ih0"], inputs["w_hh0"], inputs["b0"])
    role1 = _prep_role(inputs["w_ih1"], inputs["w_hh1"], inputs["b1"])
    xz = np.zeros((128, 4 * NITER * TB * PB), BF16NP)
    in_maps = []
    for c in range(NC):
        role = role0 if c < 4 else role1
        m = dict(role)
        m["fcwT"] = fcm
        if c < 4:
            xs = x[c * PB:(c + 1) * PB]              # [PB, T, D]
            xt = np.zeros((128, 4, NITER * TB, PB), np.float32)
            xsw = xs.reshape(PB, T, 2, 128).transpose(3, 2, 1, 0)
            xt[:, :2, :T, :] = xsw
            m["xT"] = np.ascontiguousarray(
                xt.reshape(128, 4 * NITER * TB * PB)).astype(BF16NP)
        else:
            m["xT"] = xz
        in_maps.append(m)
    res = run_bass_kernel_spmd(nc, in_maps, core_ids=list(range(NC)), **kw)
    outp = np.zeros((2 * B, 1), np.float32)
    for c in range(NC):
        r = res.results[c]["out"]
        if c < 4:
            outp[c * PB:(c + 1) * PB] = r[:PB]       # layer-0 hn rows
        else:
            cc = c - 4
            outp[B + cc * PB:B + (cc + 1) * PB] = r[PB:]  # layer-1 rows
    return outp, res


def kernel(**inputs):
    outp, _ = run(inputs)
    return outp
